# revision 25
# baseline (speedup 1.0000x reference)
"""Trainium2 Bass kernel for nn_Attn (additive/Bahdanau-style attention).

Math (per batch b):
    Wh, We   = W[:, :D], W[:, D:]                       # [D,D] each
    energy   = tanh(enc @ We.T + hidden @ Wh.T + b)     # [S, D]
    scores   = energy @ v, masked to length, softmax    # [S]
    context  = scores @ enc                             # [D]

Sharding: data-parallel over batch B=16 across 8 cores (2 batches/core);
W, b, v replicated.

Device-side layout choices (prepared host-side, pure relayout of inputs):
  - encT  [BL, D, S]: enc transposed, so the contraction dim d lands on SBUF
    partitions for the pass-1 matmuls (PE contracts along partitions).
  - enc   [BL, S, D]: natural layout for the pass-2 (context) matmuls.
  - wt    [2D, D] = W.T: rows 0:D = Wh^T [d,e], rows D:2D = We^T [d,e].
  - hidT  [D, BL], bcol/vcol [128, D/128]: chunk-column layouts.

All heavy matmuls run as float32r (full-rate fp32 mode on the PE for
moving-dim >= 256).  Pass-1 computes energy^T tiles [e=128, s=512] so the
per-batch bias (hid_proj + b) is a per-partition scalar for the ACT tanh,
and the v-dot is a PE partition-reduction.  Softmax is done on a [1, S]
row (mask via iota/is_ge, fused add+max, fused exp+sum), normalization is
folded into the output scale.
"""

import numpy as np

B, S, D = 16, 2048, 1024
NCORES = 8
BL = B // NCORES

_NC_CACHE = {}


def _build_program(bl, s, d, st, stage="all"):
    import concourse.bacc as bacc
    import concourse.bass as bass
    import concourse.mybir as mybir
    import concourse.tile as tile

    f32 = mybir.dt.float32
    f32r = mybir.dt.float32r
    i32 = mybir.dt.int32
    Tanh = mybir.ActivationFunctionType.Tanh
    Exp = mybir.ActivationFunctionType.Exp
    Alu = mybir.AluOpType

    dc = d // 128      # contraction chunks
    ns = s // st       # pass-1 s-tiles
    sc2 = s // 128     # pass-2 s-chunks
    NEG_BIG = -1.0e30

    nc = bacc.Bacc()
    scratch_d = nc.dram_tensor("attn_scratch", [bl, s], f32)
    scratch2_d = nc.dram_tensor("hp_scratch", [bl, d], f32)
    encT_d = nc.declare_dram_parameter("encT", [bl, d, s], f32, isOutput=False)
    enc_d = nc.declare_dram_parameter("enc", [bl, s, d], f32, isOutput=False)
    wt_d = nc.declare_dram_parameter("wt", [2 * d, d], f32, isOutput=False)
    hidT_d = nc.declare_dram_parameter("hidT", [d, bl], f32, isOutput=False)
    bcol_d = nc.declare_dram_parameter("bcol", [128, dc], f32, isOutput=False)
    vcol_d = nc.declare_dram_parameter("vcol", [128, dc], f32, isOutput=False)
    len_d = nc.declare_dram_parameter("len_i", [1, bl], i32, isOutput=False)
    if stage == "all":
        out_d = nc.declare_dram_parameter("ctx_out", [bl, d], f32, isOutput=True)
    else:
        out_d = nc.declare_dram_parameter("ctx_out", [bl, s], f32, isOutput=True)

    with tile.TileContext(nc) as tc:
        with (
            tc.tile_pool(name="consts", bufs=1) as consts,
            tc.tile_pool(name="etp", bufs=3) as etp,
            tc.tile_pool(name="enp", bufs=4) as enp,
            tc.tile_pool(name="p2p", bufs=8) as p2p,
            tc.tile_pool(name="sb1", bufs=1) as sb1,
            tc.tile_pool(name="psA", bufs=3, space="PSUM") as psA,
            tc.tile_pool(name="psS", bufs=1, space="PSUM") as psS,
            tc.tile_pool(name="psH", bufs=1, space="PSUM") as psH,
            tc.tile_pool(name="psM", bufs=2, space="PSUM") as psM,
        ):
            # ------------- constants -------------
            # DMA emission order matters at startup (~10 MiB must stream in
            # before steady state): hidT+Wh^T first (they gate the bias that
            # the first tanh needs), then the first encT s-tile, then We^T
            # chunk-by-chunk just-in-time for the pass-1 K-loop.
            # float32r tiles: the BIR verifier requires fp32r matmul
            # operands to be produced as fp32r, so tiles feeding the PE are
            # declared f32r and the DRAM side of each DMA is bitcast.
            hidT_sb = consts.tile([128, dc, bl], f32r)
            nc.sync.dma_start(
                out=hidT_sb,
                in_=hidT_d.rearrange("(c p) b -> p c b", p=128).bitcast(f32r),
            )
            bcol_sb = consts.tile([128, dc], f32)
            nc.sync.dma_start(out=bcol_sb, in_=bcol_d[:, :])
            # Wh^T chunks overlay the pass-2 pool: used only for hid_proj at
            # the start, then the slots recycle into en2 tiles.
            whT_tiles = []
            for c in range(dc):
                wh = p2p.tile([128, d], f32r, tag="en2", name=f"whT{c}")
                nc.sync.dma_start(
                    out=wh, in_=wt_d[c * 128:(c + 1) * 128, :].bitcast(f32r)
                )
                whT_tiles.append(wh)
            # First encT s-tile, prefetched ahead of the We^T stream.
            pre_et = {}
            et0 = etp.tile([128, dc, st], f32r, tag="et", name="et_pre")
            nc.sync.dma_start(
                out=et0,
                in_=encT_d[0, :, 0:st].rearrange("(c p) x -> p c x", p=128)
                .bitcast(f32r),
            )
            pre_et[(0, 0)] = et0
            wt_sb = consts.tile([128, dc, d], f32r)   # We^T chunks
            for c in range(dc):
                nc.sync.dma_start(
                    out=wt_sb[:, c, :],
                    in_=wt_d[(dc + c) * 128:(dc + c + 1) * 128, :].bitcast(f32r),
                )
            if ns > 1:
                et1 = etp.tile([128, dc, st], f32r, tag="et", name="et_pre1")
                nc.sync.dma_start(
                    out=et1,
                    in_=encT_d[0, :, st:2 * st].rearrange("(c p) x -> p c x", p=128)
                    .bitcast(f32r),
                )
                pre_et[(0, 1)] = et1
            vcol_sb = consts.tile([128, dc], f32)
            nc.sync.dma_start(out=vcol_sb, in_=vcol_d[:, :])
            len_i_sb = consts.tile([1, bl], i32)
            nc.sync.dma_start(out=len_i_sb, in_=len_d[:, :])
            len_f_sb = consts.tile([1, bl], f32)
            nc.vector.tensor_copy(len_f_sb, len_i_sb)
            iota_i = consts.tile([1, s], i32)
            nc.gpsimd.iota(iota_i, pattern=[[1, s]], base=0, channel_multiplier=0)
            iota_f = consts.tile([1, s], f32)
            nc.vector.tensor_copy(iota_f, iota_i)
            ones_sb = consts.tile([128, 1], f32)
            nc.vector.memset(ones_sb, 1.0)
            # Upper bound M = sum|v| >= any score (|tanh|<=1), used instead
            # of the true max in softmax -- removes the serial max-reduce.
            vabs = consts.tile([128, 1], f32)
            nc.vector.reduce_sum(
                out=vabs, in_=vcol_sb, axis=mybir.AxisListType.X,
                apply_absolute_value=True,
            )
            psv = psS.tile([1, st], f32, tag="s", name="psv")
            nc.tensor.matmul(psv[:, 0:1], ones_sb[:, 0:1], vabs, start=True, stop=True)
            negM = consts.tile([1, 1], f32)
            nc.scalar.mul(negM, psv[:, 0:1], -1.0)
            # valid01[bb] = (iota < len[bb]) as {1.0, 0.0}; independent of
            # scores, so computed up front while DVE is idle.
            valid01 = []
            for b_ in range(bl):
                vv = sb1.tile([1, s], f32, tag=f"valid{b_}", name=f"valid{b_}")
                nc.vector.tensor_scalar(
                    vv, iota_f, len_f_sb[0:1, b_:b_ + 1], None, op0=Alu.is_lt
                )
                valid01.append(vv)

            # ------------- hid_proj + b  ->  bias_all[e_chunk][:, b] -------------
            # hidT-stationary (tiny weight loads), kc-outer so each matmul
            # only needs Wh^T chunk kc as the DMA delivers it.  One
            # accumulation group per 512-wide PSUM bank half (start=True
            # clears has_written for the WHOLE bank, so groups must not
            # interleave within a bank).  Output is [b, e]; bounce through
            # DRAM to get the [e-partition] layout the tanh bias needs.
            nh2 = max(1, d // 512)
            hwb = d // nh2
            ps_hb = psH.tile([bl, d], f32)
            for kc in range(dc):
                for h in range(nh2):
                    nc.tensor.matmul(
                        ps_hb[:, h * hwb:(h + 1) * hwb],
                        hidT_sb[:, kc, :],
                        whT_tiles[kc][:, h * hwb:(h + 1) * hwb],
                        start=(kc == 0),
                        stop=(kc == dc - 1),
                        skip_group_check=True,
                    )
            hp_sb = consts.tile([bl, d], f32)
            nc.scalar.copy(hp_sb, ps_hb)
            nc.gpsimd.dma_start(out=scratch2_d[:, :], in_=hp_sb)
            bias_raw = consts.tile([128, dc, bl], f32)
            for b_ in range(bl):
                nc.gpsimd.dma_start(
                    out=bias_raw[:, :, b_],
                    in_=scratch2_d[b_, :].rearrange("(c p) -> p c", p=128),
                )
            # On ACT (not DVE tensor_scalar): the TensorScalar ISA struct has
            # a single sync-wait slot, and this op needs PE + DMA waits.
            Identity = mybir.ActivationFunctionType.Identity
            bias_all = consts.tile([128, dc, bl], f32)
            for ec in range(dc):
                nc.scalar.activation(
                    bias_all[:, ec, :],
                    bias_raw[:, ec, :],
                    Identity,
                    bias=bcol_sb[:, ec:ec + 1],
                )

            def flush_pending(pending):
                # Emit the deferred partition-reduce + copy for the previous
                # s-tile; deferring gives the DVE v-dot chain time to finish
                # without stalling the PE.
                acc_p, sco_p, sl_p = pending
                sps = psS.tile([1, st], f32, tag="s")
                nc.tensor.matmul(
                    sps, ones_sb[:, 0:1], acc_p, start=True, stop=True
                )
                nc.vector.tensor_copy(sco_p[:, sl_p], sps)

            pending = None
            for bb in range(bl):
                # ------------- pass 1: scores -------------
                scores_sb = sb1.tile([1, s], f32, tag="scores")
                for sti in range(ns):
                    et = pre_et.pop((bb, sti), None)
                    if et is None:
                        et = etp.tile([128, dc, st], f32r, tag="et")
                        nc.sync.dma_start(
                            out=et,
                            in_=encT_d[bb, :, sti * st:(sti + 1) * st].rearrange(
                                "(c p) x -> p c x", p=128
                            ).bitcast(f32r),
                        )
                    acc = enp.tile([128, st], f32, tag="acc")
                    for ec in range(dc):
                        ps = psA.tile([128, st], f32, tag="proj")
                        for kc in range(dc):
                            nc.tensor.matmul(
                                ps,
                                wt_sb[:, kc, ec * 128:(ec + 1) * 128],
                                et[:, kc, :],
                                start=(kc == 0),
                                stop=(kc == dc - 1),
                            )
                        if ec == min(2, dc - 1) and pending is not None:
                            flush_pending(pending)
                            pending = None
                        en = enp.tile([128, st], f32, tag="en")
                        nc.scalar.activation(
                            en, ps, Tanh, bias=bias_all[:, ec, bb:bb + 1]
                        )
                        # v-dot on DVE: acc[p, s] accumulates v[ec*128+p]*en
                        if ec == 0:
                            nc.vector.tensor_scalar_mul(
                                acc, en, vcol_sb[:, 0:1]
                            )
                        else:
                            nc.vector.scalar_tensor_tensor(
                                acc,
                                en,
                                vcol_sb[:, ec:ec + 1],
                                acc,
                                op0=Alu.mult,
                                op1=Alu.add,
                            )
                    if pending is not None:
                        flush_pending(pending)
                    pending = (acc, scores_sb, slice(sti * st, (sti + 1) * st))
                if pending is not None:
                    flush_pending(pending)
                    pending = None

                if stage == "p1":
                    nc.gpsimd.dma_start(out=out_d[bb:bb + 1, :], in_=scores_sb)
                    continue

                # ------------- masked softmax (normalization deferred) ---------
                # exp(score - M) with the global bound M (no max-reduce);
                # mask+sum fused into one DVE pass.
                attn_raw = sb1.tile([1, s], f32, tag="araw")
                nc.scalar.activation(
                    attn_raw, scores_sb, Exp, bias=negM[0:1, 0:1]
                )
                attn_exp = sb1.tile([1, s], f32, tag="aexp")
                ssum = sb1.tile([1, 1], f32, tag="ssum")
                nc.vector.scalar_tensor_tensor(
                    attn_exp,
                    attn_raw,
                    1.0,
                    valid01[bb],
                    op0=Alu.mult,
                    op1=Alu.mult,
                    accum_out=ssum,
                )
                if stage == "sm":
                    nc.gpsimd.dma_start(out=out_d[bb:bb + 1, :], in_=attn_exp)
                    continue
                rinv = sb1.tile([1, 1], f32, tag="rinv")
                nc.vector.reciprocal(rinv, ssum)
                # [1, s] -> [128, s/128] transpose via a DRAM bounce (the
                # direct SBUF->SBUF rearrange is not AP-balanceable).
                nc.gpsimd.dma_start(out=scratch_d[bb:bb + 1, :], in_=attn_exp)
                attnT = sb1.tile([128, sc2], f32r, tag="attnT")
                nc.gpsimd.dma_start(
                    out=attnT,
                    in_=scratch_d[bb, :].rearrange("(f p) -> p f", p=128).bitcast(f32r),
                )

                # ------------- pass 2: context -------------
                nh = 2 if d > 512 else 1
                hw_ = d // nh
                halves = [psM.tile([1, hw_], f32, tag="m", name=f"cps{h}")
                          for h in range(nh)]
                for sci in range(sc2):
                    en2 = p2p.tile([128, d], f32r, tag="en2")
                    nc.sync.dma_start(
                        out=en2,
                        in_=enc_d[bb, sci * 128:(sci + 1) * 128, :].bitcast(f32r),
                    )
                    for h in range(nh):
                        nc.tensor.matmul(
                            halves[h],
                            attnT[:, sci:sci + 1],
                            en2[:, h * hw_:(h + 1) * hw_],
                            start=(sci == 0),
                            stop=(sci == sc2 - 1),
                        )
                ctx_sb = sb1.tile([1, d], f32, tag="ctx")
                for h in range(nh):
                    nc.scalar.mul(
                        ctx_sb[:, h * hw_:(h + 1) * hw_], halves[h],
                        rinv[0:1, 0:1],
                    )
                nc.gpsimd.dma_start(out=out_d[bb:bb + 1, :], in_=ctx_sb)

    nc.compile()
    return nc


def _get_nc(bl=BL, s=S, d=D, st=512, stage="all"):
    key = (bl, s, d, st, stage)
    if key not in _NC_CACHE:
        _NC_CACHE[key] = _build_program(bl, s, d, st, stage)
    return _NC_CACHE[key]


def _make_in_maps(encoder_outputs, hidden, lengths, W, b, v):
    enc = np.asarray(encoder_outputs, dtype=np.float32)
    hid = np.asarray(hidden, dtype=np.float32)
    len_ = np.asarray(lengths, dtype=np.int32)
    Wn = np.asarray(W, dtype=np.float32)
    bn = np.asarray(b, dtype=np.float32)
    vn = np.asarray(v, dtype=np.float32)

    dc = D // 128
    wt = np.ascontiguousarray(Wn.T)                      # [2D, D]
    bcol = np.ascontiguousarray(bn.reshape(dc, 128).T)   # [128, dc]
    vcol = np.ascontiguousarray(vn.reshape(dc, 128).T)
    in_maps = []
    for i in range(NCORES):
        sl = slice(BL * i, BL * (i + 1))
        e = enc[sl]
        in_maps.append(
            dict(
                encT=np.ascontiguousarray(e.transpose(0, 2, 1)),
                enc=np.ascontiguousarray(e),
                wt=wt,
                hidT=np.ascontiguousarray(hid[sl].T),
                bcol=bcol,
                vcol=vcol,
                len_i=np.ascontiguousarray(len_[sl].reshape(1, BL)),
            )
        )
    return in_maps


def run(inputs, trace=False):
    """Run on 8 NeuronCores; returns (output [B,1,D], BassKernelResults)."""
    from concourse.bass_utils import run_bass_kernel_spmd

    nc = _get_nc()
    in_maps = _make_in_maps(**inputs)
    r = run_bass_kernel_spmd(
        nc, in_maps, core_ids=list(range(NCORES)), trace=trace
    )
    out = np.concatenate(
        [np.asarray(r.results[i]["ctx_out"]) for i in range(NCORES)], axis=0
    )
    return out[:, None, :].astype(np.float32), r


def kernel(encoder_outputs, hidden, lengths, W, b, v):
    out, _ = run(
        dict(
            encoder_outputs=encoder_outputs,
            hidden=hidden,
            lengths=lengths,
            W=W,
            b=b,
            v=v,
        )
    )
    return out


# revision 28
# speedup vs baseline: 1.0589x; 1.0589x over previous
"""Trainium2 Bass kernel for nn_Attn (additive/Bahdanau-style attention).

Math (per batch b):
    Wh, We   = W[:, :D], W[:, D:]                       # [D,D] each
    energy   = tanh(enc @ We.T + hidden @ Wh.T + b)     # [S, D]
    scores   = energy @ v, masked to length, softmax    # [S]
    context  = scores @ enc                             # [D]

Sharding: data-parallel over batch B=16 across 8 cores (2 batches/core);
W, b, v replicated.

Device-side layout choices (prepared host-side, pure relayout of inputs):
  - encT  [BL, D, S]: enc transposed, so the contraction dim d lands on SBUF
    partitions for the pass-1 matmuls (PE contracts along partitions).
  - enc   [BL, S, D]: natural layout for the pass-2 (context) matmuls.
  - wt    [2D, D] = W.T: rows 0:D = Wh^T [d,e], rows D:2D = We^T [d,e].
  - hidT  [D, BL], bcol/vcol [128, D/128]: chunk-column layouts.

All heavy matmuls run as float32r (full-rate fp32 mode on the PE for
moving-dim >= 256).  Pass-1 computes energy^T tiles [e=128, s=512] so the
per-batch bias (hid_proj + b) is a per-partition scalar for the ACT tanh,
and the v-dot is a PE partition-reduction.  Softmax is done on a [1, S]
row (mask via iota/is_ge, fused add+max, fused exp+sum), normalization is
folded into the output scale.
"""

import numpy as np

B, S, D = 16, 2048, 1024
NCORES = 8
BL = B // NCORES

_NC_CACHE = {}


def _build_program(bl, s, d, st, stage="all"):
    import concourse.bacc as bacc
    import concourse.bass as bass
    import concourse.mybir as mybir
    import concourse.tile as tile

    f32 = mybir.dt.float32
    f32r = mybir.dt.float32r
    i32 = mybir.dt.int32
    Tanh = mybir.ActivationFunctionType.Tanh
    Exp = mybir.ActivationFunctionType.Exp
    Alu = mybir.AluOpType

    dc = d // 128      # contraction chunks
    ns = s // st       # pass-1 s-tiles
    sc2 = s // 128     # pass-2 s-chunks
    NEG_BIG = -1.0e30

    nc = bacc.Bacc()
    scratch_d = nc.dram_tensor("attn_scratch", [bl, s], f32)
    scratch2_d = nc.dram_tensor("hp_scratch", [bl, d], f32)
    encT_d = nc.declare_dram_parameter("encT", [bl, d, s], f32, isOutput=False)
    enc_d = nc.declare_dram_parameter("enc", [bl, s, d], f32, isOutput=False)
    wt_d = nc.declare_dram_parameter("wt", [2 * d, d], f32, isOutput=False)
    hidT_d = nc.declare_dram_parameter("hidT", [d, bl], f32, isOutput=False)
    bcol_d = nc.declare_dram_parameter("bcol", [128, dc], f32, isOutput=False)
    vcol_d = nc.declare_dram_parameter("vcol", [128, dc], f32, isOutput=False)
    len_d = nc.declare_dram_parameter("len_i", [128, bl], i32, isOutput=False)
    if stage == "all":
        out_d = nc.declare_dram_parameter("ctx_out", [bl, d], f32, isOutput=True)
    else:
        out_d = nc.declare_dram_parameter("ctx_out", [bl, s], f32, isOutput=True)

    with tile.TileContext(nc) as tc:
        with (
            tc.tile_pool(name="consts", bufs=1) as consts,
            tc.tile_pool(name="etp", bufs=4) as etp,
            tc.tile_pool(name="enp", bufs=4) as enp,
            tc.tile_pool(name="p2p", bufs=14) as p2p,
            tc.tile_pool(name="sb1", bufs=1) as sb1,
            tc.tile_pool(name="psA", bufs=3, space="PSUM") as psA,
            tc.tile_pool(name="psS", bufs=1, space="PSUM") as psS,
            tc.tile_pool(name="psH", bufs=1, space="PSUM") as psH,
            tc.tile_pool(name="psM", bufs=2, space="PSUM") as psM,
        ):
            # ------------- constants -------------
            # DMA emission order matters at startup (~10 MiB must stream in
            # before steady state): hidT+Wh^T first (they gate the bias that
            # the first tanh needs), then the first encT s-tile, then We^T
            # chunk-by-chunk just-in-time for the pass-1 K-loop.
            # float32r tiles: the BIR verifier requires fp32r matmul
            # operands to be produced as fp32r, so tiles feeding the PE are
            # declared f32r and the DRAM side of each DMA is bitcast.
            hidT_sb = consts.tile([128, dc, bl], f32r)
            nc.sync.dma_start(
                out=hidT_sb,
                in_=hidT_d.rearrange("(c p) b -> p c b", p=128).bitcast(f32r),
            )
            bcol_sb = consts.tile([128, dc], f32)
            nc.sync.dma_start(out=bcol_sb, in_=bcol_d[:, :])
            # Wh^T chunks overlay the pass-2 pool: used only for hid_proj at
            # the start, then the slots recycle into en2 tiles.
            whT_tiles = []
            for c in range(dc):
                wh = p2p.tile([128, d], f32r, tag="en2", name=f"whT{c}")
                nc.sync.dma_start(
                    out=wh, in_=wt_d[c * 128:(c + 1) * 128, :].bitcast(f32r)
                )
                whT_tiles.append(wh)
            # First encT s-tile, prefetched ahead of the We^T stream.
            pre_et = {}
            et0 = etp.tile([128, dc, st], f32r, tag="et", name="et_pre")
            nc.sync.dma_start(
                out=et0,
                in_=encT_d[0, :, 0:st].rearrange("(c p) x -> p c x", p=128)
                .bitcast(f32r),
            )
            pre_et[(0, 0)] = et0
            wt_sb = consts.tile([128, dc, d], f32r)   # We^T chunks
            for c in range(dc):
                nc.sync.dma_start(
                    out=wt_sb[:, c, :],
                    in_=wt_d[(dc + c) * 128:(dc + c + 1) * 128, :].bitcast(f32r),
                )
            if ns > 1:
                et1 = etp.tile([128, dc, st], f32r, tag="et", name="et_pre1")
                nc.sync.dma_start(
                    out=et1,
                    in_=encT_d[0, :, st:2 * st].rearrange("(c p) x -> p c x", p=128)
                    .bitcast(f32r),
                )
                pre_et[(0, 1)] = et1
            if ns > 2:
                et2 = etp.tile([128, dc, st], f32r, tag="et", name="et_pre2")
                nc.sync.dma_start(
                    out=et2,
                    in_=encT_d[0, :, 2 * st:3 * st].rearrange(
                        "(c p) x -> p c x", p=128
                    ).bitcast(f32r),
                )
                pre_et[(0, 2)] = et2
            vcol_sb = consts.tile([128, dc], f32)
            nc.sync.dma_start(out=vcol_sb, in_=vcol_d[:, :])
            len_i_sb = consts.tile([128, bl], i32)
            nc.sync.dma_start(out=len_i_sb, in_=len_d[:, :])
            len_f_sb = consts.tile([128, bl], f32)
            nc.vector.tensor_copy(len_f_sb, len_i_sb)
            # Everything score-related lives in [128(p), sc2(f)] layout with
            # s = f*128 + p, so softmax ops use all 128 lanes and the
            # pass-2 stationary operand needs no transpose.
            iotaT_i = consts.tile([128, sc2], i32)
            nc.gpsimd.iota(
                iotaT_i, pattern=[[128, sc2]], base=0, channel_multiplier=1
            )
            iotaT_f = consts.tile([128, sc2], f32)
            nc.vector.tensor_copy(iotaT_f, iotaT_i)
            ones_sb = consts.tile([128, 1], f32)
            nc.vector.memset(ones_sb, 1.0)
            ones_row = consts.tile([1, 128], f32)
            nc.vector.memset(ones_row, 1.0)
            # Upper bound M = sum|v| >= any score (|tanh|<=1), used instead
            # of the true max in softmax -- removes the serial max-reduce.
            vabs = consts.tile([128, 1], f32)
            nc.vector.reduce_sum(
                out=vabs, in_=vcol_sb, axis=mybir.AxisListType.X,
                apply_absolute_value=True,
            )
            psv = psS.tile([1, st], f32, tag="s", name="psv")
            nc.tensor.matmul(psv[:, 0:1], ones_sb[:, 0:1], vabs, start=True, stop=True)
            mtot = consts.tile([1, 1], f32)
            nc.vector.tensor_copy(mtot, psv[:, 0:1])
            # broadcast -M to all 128 partitions via a K=1 matmul
            psb = psS.tile([128, 1], f32, tag="s", name="psb")
            nc.tensor.matmul(psb, ones_row[:, :], mtot[:, :], start=True, stop=True)
            negM_bc = consts.tile([128, 1], f32)
            nc.scalar.mul(negM_bc, psb, -1.0)
            validT = []
            for b_ in range(bl):
                vv = consts.tile([128, sc2], f32, name=f"validT{b_}")
                nc.vector.tensor_scalar(
                    vv, iotaT_f, len_f_sb[:, b_:b_ + 1], None, op0=Alu.is_lt
                )
                validT.append(vv)

            # ------------- hid_proj + b  ->  bias_all[e_chunk][:, b] -------------
            # hidT-stationary (tiny weight loads), kc-outer so each matmul
            # only needs Wh^T chunk kc as the DMA delivers it.  One
            # accumulation group per 512-wide PSUM bank half (start=True
            # clears has_written for the WHOLE bank, so groups must not
            # interleave within a bank).  Output is [b, e]; bounce through
            # DRAM to get the [e-partition] layout the tanh bias needs.
            nh2 = max(1, d // 512)
            hwb = d // nh2
            ps_hb = psH.tile([bl, d], f32)
            for kc in range(dc):
                for h in range(nh2):
                    nc.tensor.matmul(
                        ps_hb[:, h * hwb:(h + 1) * hwb],
                        hidT_sb[:, kc, :],
                        whT_tiles[kc][:, h * hwb:(h + 1) * hwb],
                        start=(kc == 0),
                        stop=(kc == dc - 1),
                        skip_group_check=True,
                    )
            hp_sb = consts.tile([bl, d], f32)
            nc.scalar.copy(hp_sb, ps_hb)
            nc.gpsimd.dma_start(out=scratch2_d[:, :], in_=hp_sb)
            bias_raw = consts.tile([128, dc, bl], f32)
            for b_ in range(bl):
                nc.gpsimd.dma_start(
                    out=bias_raw[:, :, b_],
                    in_=scratch2_d[b_, :].rearrange("(c p) -> p c", p=128),
                )
            # On ACT (not DVE tensor_scalar): the TensorScalar ISA struct has
            # a single sync-wait slot, and this op needs PE + DMA waits.
            Identity = mybir.ActivationFunctionType.Identity
            bias_all = consts.tile([128, dc, bl], f32)
            for ec in range(dc):
                nc.scalar.activation(
                    bias_all[:, ec, :],
                    bias_raw[:, ec, :],
                    Identity,
                    bias=bcol_sb[:, ec:ec + 1],
                )

            nst = st // 128   # 128-wide score chunks per s-tile

            def flush_pending(pending):
                # Emit the deferred partition-reduces + copies for the
                # previous s-tile; deferring gives the DVE v-dot chain time
                # to finish without stalling the PE.  Each chunk c of acc
                # column-sums into scoresT[:, f] (s = f*128 + p).
                acc_p, sco_p, sti_p = pending
                for c_ in range(nst):
                    sps = psS.tile([128, 1], f32, tag="s")
                    nc.tensor.matmul(
                        sps,
                        acc_p[:, c_ * 128:(c_ + 1) * 128],
                        ones_sb[:, 0:1],
                        start=True,
                        stop=True,
                    )
                    nc.vector.tensor_copy(
                        sco_p[:, sti_p * nst + c_:sti_p * nst + c_ + 1], sps
                    )

            pending = None
            for bb in range(bl):
                # ------------- pass 1: scores -------------
                scores_sb = sb1.tile([128, sc2], f32, tag="scores")
                for sti in range(ns):
                    et = pre_et.pop((bb, sti), None)
                    if et is None:
                        et = etp.tile([128, dc, st], f32r, tag="et")
                        nc.sync.dma_start(
                            out=et,
                            in_=encT_d[bb, :, sti * st:(sti + 1) * st].rearrange(
                                "(c p) x -> p c x", p=128
                            ).bitcast(f32r),
                        )
                    acc = enp.tile([128, st], f32, tag="acc")
                    for ec in range(dc):
                        ps = psA.tile([128, st], f32, tag="proj")
                        for kc in range(dc):
                            nc.tensor.matmul(
                                ps,
                                wt_sb[:, kc, ec * 128:(ec + 1) * 128],
                                et[:, kc, :],
                                start=(kc == 0),
                                stop=(kc == dc - 1),
                            )
                        if ec == min(2, dc - 1) and pending is not None:
                            flush_pending(pending)
                            pending = None
                        en = enp.tile([128, st], f32, tag="en")
                        nc.scalar.activation(
                            en, ps, Tanh, bias=bias_all[:, ec, bb:bb + 1]
                        )
                        # v-dot on DVE: acc[p, s] accumulates v[ec*128+p]*en
                        if ec == 0:
                            nc.vector.tensor_scalar_mul(
                                acc, en, vcol_sb[:, 0:1]
                            )
                        else:
                            nc.vector.scalar_tensor_tensor(
                                acc,
                                en,
                                vcol_sb[:, ec:ec + 1],
                                acc,
                                op0=Alu.mult,
                                op1=Alu.add,
                            )
                    if pending is not None:
                        flush_pending(pending)
                    pending = (acc, scores_sb, sti)
                if pending is not None:
                    flush_pending(pending)
                    pending = None

                if stage == "p1":
                    nc.gpsimd.dma_start(
                        out=out_d[bb, :].rearrange("(f p) -> p f", p=128),
                        in_=scores_sb,
                    )
                    continue

                # ------------- masked softmax (normalization deferred) ---------
                # exp(score - M) with the global bound M = sum|v| (no
                # max-reduce); mask + per-partition row-sum fused in one
                # DVE pass; all ops are [128, sc2] so they cost ~100 ns.
                attn_raw = sb1.tile([128, sc2], f32, tag="araw")
                nc.scalar.activation(
                    attn_raw, scores_sb, Exp, bias=negM_bc[:, 0:1]
                )
                attn_exp = sb1.tile([128, sc2], f32, tag="aexp")
                psums = sb1.tile([128, 1], f32, tag="psums")
                nc.vector.scalar_tensor_tensor(
                    attn_exp,
                    attn_raw,
                    1.0,
                    validT[bb],
                    op0=Alu.mult,
                    op1=Alu.mult,
                    accum_out=psums,
                )
                # attnT (f32r) is just a rounding copy -- no transpose needed
                attnT = sb1.tile([128, sc2], f32r, tag="attnT")
                nc.scalar.copy(attnT, attn_exp)
                # total sum across partitions -> reciprocal
                psm = psS.tile([128, 1], f32, tag="s", name="psm")
                nc.tensor.matmul(
                    psm[0:1, 0:1], psums, ones_sb[:, 0:1], start=True, stop=True
                )
                if stage == "sm":
                    nc.gpsimd.dma_start(
                        out=out_d[bb, :].rearrange("(f p) -> p f", p=128),
                        in_=attn_exp,
                    )
                    continue
                rinv = sb1.tile([1, 1], f32, tag="rinv")
                nc.vector.reciprocal(rinv, psm[0:1, 0:1])

                # ------------- pass 2: context -------------
                nh = 2 if d > 512 else 1
                hw_ = d // nh
                halves = [psM.tile([1, hw_], f32, tag="m", name=f"cps{h}")
                          for h in range(nh)]
                for sci in range(sc2):
                    en2 = p2p.tile([128, d], f32r, tag="en2")
                    nc.sync.dma_start(
                        out=en2,
                        in_=enc_d[bb, sci * 128:(sci + 1) * 128, :].bitcast(f32r),
                    )
                    for h in range(nh):
                        nc.tensor.matmul(
                            halves[h],
                            attnT[:, sci:sci + 1],
                            en2[:, h * hw_:(h + 1) * hw_],
                            start=(sci == 0),
                            stop=(sci == sc2 - 1),
                        )
                ctx_sb = sb1.tile([1, d], f32, tag="ctx")
                for h in range(nh):
                    nc.scalar.mul(
                        ctx_sb[:, h * hw_:(h + 1) * hw_], halves[h],
                        rinv[0:1, 0:1],
                    )
                nc.gpsimd.dma_start(out=out_d[bb:bb + 1, :], in_=ctx_sb)

    nc.compile()
    return nc


def _get_nc(bl=BL, s=S, d=D, st=512, stage="all"):
    key = (bl, s, d, st, stage)
    if key not in _NC_CACHE:
        _NC_CACHE[key] = _build_program(bl, s, d, st, stage)
    return _NC_CACHE[key]


def _make_in_maps(encoder_outputs, hidden, lengths, W, b, v):
    enc = np.asarray(encoder_outputs, dtype=np.float32)
    hid = np.asarray(hidden, dtype=np.float32)
    len_ = np.asarray(lengths, dtype=np.int32)
    Wn = np.asarray(W, dtype=np.float32)
    bn = np.asarray(b, dtype=np.float32)
    vn = np.asarray(v, dtype=np.float32)

    dc = D // 128
    wt = np.ascontiguousarray(Wn.T)                      # [2D, D]
    bcol = np.ascontiguousarray(bn.reshape(dc, 128).T)   # [128, dc]
    vcol = np.ascontiguousarray(vn.reshape(dc, 128).T)
    in_maps = []
    for i in range(NCORES):
        sl = slice(BL * i, BL * (i + 1))
        e = enc[sl]
        in_maps.append(
            dict(
                encT=np.ascontiguousarray(e.transpose(0, 2, 1)),
                enc=np.ascontiguousarray(e),
                wt=wt,
                hidT=np.ascontiguousarray(hid[sl].T),
                bcol=bcol,
                vcol=vcol,
                len_i=np.ascontiguousarray(
                    np.broadcast_to(len_[sl].reshape(1, BL), (128, BL)).copy()
                ),
            )
        )
    return in_maps


def run(inputs, trace=False):
    """Run on 8 NeuronCores; returns (output [B,1,D], BassKernelResults)."""
    from concourse.bass_utils import run_bass_kernel_spmd

    nc = _get_nc()
    in_maps = _make_in_maps(**inputs)
    r = run_bass_kernel_spmd(
        nc, in_maps, core_ids=list(range(NCORES)), trace=trace
    )
    out = np.concatenate(
        [np.asarray(r.results[i]["ctx_out"]) for i in range(NCORES)], axis=0
    )
    return out[:, None, :].astype(np.float32), r


def kernel(encoder_outputs, hidden, lengths, W, b, v):
    out, _ = run(
        dict(
            encoder_outputs=encoder_outputs,
            hidden=hidden,
            lengths=lengths,
            W=W,
            b=b,
            v=v,
        )
    )
    return out


# revision 29
# speedup vs baseline: 1.0992x; 1.0380x over previous
"""Trainium2 Bass kernel for nn_Attn (additive/Bahdanau-style attention).

Math (per batch b):
    Wh, We   = W[:, :D], W[:, D:]                       # [D,D] each
    energy   = tanh(enc @ We.T + hidden @ Wh.T + b)     # [S, D]
    scores   = energy @ v, masked to length, softmax    # [S]
    context  = scores @ enc                             # [D]

Sharding: data-parallel over batch B=16 across 8 cores (2 batches/core);
W, b, v replicated.

Device-side layout choices (prepared host-side, pure relayout of inputs):
  - encT  [BL, D, S]: enc transposed, so the contraction dim d lands on SBUF
    partitions for the pass-1 matmuls (PE contracts along partitions).
  - enc   [BL, S, D]: natural layout for the pass-2 (context) matmuls.
  - wt    [2D, D] = W.T: rows 0:D = Wh^T [d,e], rows D:2D = We^T [d,e].
  - hidT  [D, BL], bcol/vcol [128, D/128]: chunk-column layouts.

All heavy matmuls run as float32r (full-rate fp32 mode on the PE for
moving-dim >= 256).  Pass-1 computes energy^T tiles [e=128, s=512] so the
per-batch bias (hid_proj + b) is a per-partition scalar for the ACT tanh,
and the v-dot is a PE partition-reduction.  Softmax is done on a [1, S]
row (mask via iota/is_ge, fused add+max, fused exp+sum), normalization is
folded into the output scale.
"""

import numpy as np

B, S, D = 16, 2048, 1024
NCORES = 8
BL = B // NCORES

_NC_CACHE = {}


def _build_program(bl, s, d, st, stage="all"):
    import concourse.bacc as bacc
    import concourse.bass as bass
    import concourse.mybir as mybir
    import concourse.tile as tile

    f32 = mybir.dt.float32
    f32r = mybir.dt.float32r
    i32 = mybir.dt.int32
    Tanh = mybir.ActivationFunctionType.Tanh
    Exp = mybir.ActivationFunctionType.Exp
    Alu = mybir.AluOpType

    dc = d // 128      # contraction chunks
    ns = s // st       # pass-1 s-tiles
    sc2 = s // 128     # pass-2 s-chunks
    NEG_BIG = -1.0e30

    nc = bacc.Bacc()
    scratch_d = nc.dram_tensor("attn_scratch", [bl, s], f32)
    scratch2_d = nc.dram_tensor("hp_scratch", [bl, d], f32)
    encT_d = nc.declare_dram_parameter("encT", [bl, d, s], f32, isOutput=False)
    enc_d = nc.declare_dram_parameter("enc", [bl, s, d], f32, isOutput=False)
    wt_d = nc.declare_dram_parameter("wt", [2 * d, d], f32, isOutput=False)
    hidT_d = nc.declare_dram_parameter("hidT", [d, bl], f32, isOutput=False)
    bcol_d = nc.declare_dram_parameter("bcol", [128, dc], f32, isOutput=False)
    vcol_d = nc.declare_dram_parameter("vcol", [128, dc], f32, isOutput=False)
    len_d = nc.declare_dram_parameter("len_i", [128, bl], i32, isOutput=False)
    if stage == "all":
        out_d = nc.declare_dram_parameter("ctx_out", [bl, d], f32, isOutput=True)
    else:
        out_d = nc.declare_dram_parameter("ctx_out", [bl, s], f32, isOutput=True)

    with tile.TileContext(nc) as tc:
        with (
            tc.tile_pool(name="consts", bufs=1) as consts,
            tc.tile_pool(name="etp", bufs=4) as etp,
            tc.tile_pool(name="enp", bufs=4) as enp,
            tc.tile_pool(name="p2p", bufs=14) as p2p,
            tc.tile_pool(name="sb1", bufs=1) as sb1,
            tc.tile_pool(name="psA", bufs=4, space="PSUM") as psA,
            tc.tile_pool(name="psS", bufs=2, space="PSUM") as psS,
            tc.tile_pool(name="psM", bufs=1, space="PSUM") as psM,
        ):
            # ------------- constants -------------
            # DMA emission order matters at startup (~10 MiB must stream in
            # before steady state): hidT+Wh^T first (they gate the bias that
            # the first tanh needs), then the first encT s-tile, then We^T
            # chunk-by-chunk just-in-time for the pass-1 K-loop.
            # float32r tiles: the BIR verifier requires fp32r matmul
            # operands to be produced as fp32r, so tiles feeding the PE are
            # declared f32r and the DRAM side of each DMA is bitcast.
            hidT_sb = consts.tile([128, dc, bl], f32r)
            nc.sync.dma_start(
                out=hidT_sb,
                in_=hidT_d.rearrange("(c p) b -> p c b", p=128).bitcast(f32r),
            )
            bcol_sb = consts.tile([128, dc], f32)
            nc.sync.dma_start(out=bcol_sb, in_=bcol_d[:, :])
            # Wh^T chunks overlay the pass-2 pool: used only for hid_proj at
            # the start, then the slots recycle into en2 tiles.
            whT_tiles = []
            for c in range(dc):
                wh = p2p.tile([128, d], f32r, tag="en2", name=f"whT{c}")
                nc.sync.dma_start(
                    out=wh, in_=wt_d[c * 128:(c + 1) * 128, :].bitcast(f32r)
                )
                whT_tiles.append(wh)
            # First encT s-tile, prefetched ahead of the We^T stream.
            pre_et = {}
            et0 = etp.tile([128, dc, st], f32r, tag="et", name="et_pre")
            nc.sync.dma_start(
                out=et0,
                in_=encT_d[0, :, 0:st].rearrange("(c p) x -> p c x", p=128)
                .bitcast(f32r),
            )
            pre_et[(0, 0)] = et0
            wt_sb = consts.tile([128, dc, d], f32r)   # We^T chunks
            for c in range(dc):
                nc.sync.dma_start(
                    out=wt_sb[:, c, :],
                    in_=wt_d[(dc + c) * 128:(dc + c + 1) * 128, :].bitcast(f32r),
                )
            if ns > 1:
                et1 = etp.tile([128, dc, st], f32r, tag="et", name="et_pre1")
                nc.sync.dma_start(
                    out=et1,
                    in_=encT_d[0, :, st:2 * st].rearrange("(c p) x -> p c x", p=128)
                    .bitcast(f32r),
                )
                pre_et[(0, 1)] = et1
            if ns > 2:
                et2 = etp.tile([128, dc, st], f32r, tag="et", name="et_pre2")
                nc.sync.dma_start(
                    out=et2,
                    in_=encT_d[0, :, 2 * st:3 * st].rearrange(
                        "(c p) x -> p c x", p=128
                    ).bitcast(f32r),
                )
                pre_et[(0, 2)] = et2
            vcol_sb = consts.tile([128, dc], f32)
            nc.sync.dma_start(out=vcol_sb, in_=vcol_d[:, :])
            len_i_sb = consts.tile([128, bl], i32)
            nc.sync.dma_start(out=len_i_sb, in_=len_d[:, :])
            len_f_sb = consts.tile([128, bl], f32)
            nc.vector.tensor_copy(len_f_sb, len_i_sb)
            # Everything score-related lives in [128(p), sc2(f)] layout with
            # s = f*128 + p, so softmax ops use all 128 lanes and the
            # pass-2 stationary operand needs no transpose.
            iotaT_i = consts.tile([128, sc2], i32)
            nc.gpsimd.iota(
                iotaT_i, pattern=[[128, sc2]], base=0, channel_multiplier=1
            )
            iotaT_f = consts.tile([128, sc2], f32)
            nc.vector.tensor_copy(iotaT_f, iotaT_i)
            ones_sb = consts.tile([128, 1], f32)
            nc.vector.memset(ones_sb, 1.0)
            ones_row = consts.tile([1, 128], f32)
            nc.vector.memset(ones_row, 1.0)
            # Upper bound M = sum|v| >= any score (|tanh|<=1), used instead
            # of the true max in softmax -- removes the serial max-reduce.
            vabs = consts.tile([128, 1], f32)
            nc.vector.reduce_sum(
                out=vabs, in_=vcol_sb, axis=mybir.AxisListType.X,
                apply_absolute_value=True,
            )
            psv = psS.tile([1, st], f32, tag="s", name="psv")
            nc.tensor.matmul(psv[:, 0:1], ones_sb[:, 0:1], vabs, start=True, stop=True)
            mtot = consts.tile([1, 1], f32)
            nc.vector.tensor_copy(mtot, psv[:, 0:1])
            # broadcast -M to all 128 partitions via a K=1 matmul
            psb = psS.tile([128, 1], f32, tag="s", name="psb")
            nc.tensor.matmul(psb, ones_row[:, :], mtot[:, :], start=True, stop=True)
            negM_bc = consts.tile([128, 1], f32)
            nc.scalar.mul(negM_bc, psb, -1.0)
            validT = []
            for b_ in range(bl):
                vv = consts.tile([128, sc2], f32, name=f"validT{b_}")
                nc.vector.tensor_scalar(
                    vv, iotaT_f, len_f_sb[:, b_:b_ + 1], None, op0=Alu.is_lt
                )
                validT.append(vv)

            # ------------- hid_proj + b  ->  bias_all[e_chunk][:, b] -------------
            # hidT-stationary (tiny weight loads), kc-outer so each matmul
            # only needs Wh^T chunk kc as the DMA delivers it.  One
            # accumulation group per 512-wide PSUM bank half (start=True
            # clears has_written for the WHOLE bank, so groups must not
            # interleave within a bank).  Output is [b, e]; bounce through
            # DRAM to get the [e-partition] layout the tanh bias needs.
            nh2 = max(1, d // 512)
            hwb = d // nh2
            ps_hb = psM.tile([bl, d], f32, tag="m")
            for kc in range(dc):
                for h in range(nh2):
                    nc.tensor.matmul(
                        ps_hb[:, h * hwb:(h + 1) * hwb],
                        hidT_sb[:, kc, :],
                        whT_tiles[kc][:, h * hwb:(h + 1) * hwb],
                        start=(kc == 0),
                        stop=(kc == dc - 1),
                        skip_group_check=True,
                    )
            hp_sb = consts.tile([bl, d], f32)
            nc.scalar.copy(hp_sb, ps_hb)
            nc.gpsimd.dma_start(out=scratch2_d[:, :], in_=hp_sb)
            bias_raw = consts.tile([128, dc, bl], f32)
            for b_ in range(bl):
                nc.gpsimd.dma_start(
                    out=bias_raw[:, :, b_],
                    in_=scratch2_d[b_, :].rearrange("(c p) -> p c", p=128),
                )
            # On ACT (not DVE tensor_scalar): the TensorScalar ISA struct has
            # a single sync-wait slot, and this op needs PE + DMA waits.
            Identity = mybir.ActivationFunctionType.Identity
            bias_all = consts.tile([128, dc, bl], f32)
            for ec in range(dc):
                nc.scalar.activation(
                    bias_all[:, ec, :],
                    bias_raw[:, ec, :],
                    Identity,
                    bias=bcol_sb[:, ec:ec + 1],
                )

            nst = st // 128   # 128-wide score chunks per s-tile

            def flush_pending(pending):
                # Emit the deferred partition-reduces + copies for the
                # previous s-tile; deferring gives the DVE v-dot chain time
                # to finish without stalling the PE.  Each chunk c of acc
                # column-sums into scoresT[:, f] (s = f*128 + p).
                acc_p, sco_p, sti_p = pending
                for c_ in range(nst):
                    sps = psS.tile([128, 1], f32, tag="s")
                    nc.tensor.matmul(
                        sps,
                        acc_p[:, c_ * 128:(c_ + 1) * 128],
                        ones_sb[:, 0:1],
                        start=True,
                        stop=True,
                    )
                    nc.vector.tensor_copy(
                        sco_p[:, sti_p * nst + c_:sti_p * nst + c_ + 1], sps
                    )

            pending = None
            for bb in range(bl):
                # ------------- pass 1: scores -------------
                scores_sb = sb1.tile([128, sc2], f32, tag="scores")
                for sti in range(ns):
                    et = pre_et.pop((bb, sti), None)
                    if et is None:
                        et = etp.tile([128, dc, st], f32r, tag="et")
                        nc.sync.dma_start(
                            out=et,
                            in_=encT_d[bb, :, sti * st:(sti + 1) * st].rearrange(
                                "(c p) x -> p c x", p=128
                            ).bitcast(f32r),
                        )
                    acc = enp.tile([128, st], f32, tag="acc")
                    for ec in range(dc):
                        ps = psA.tile([128, st], f32, tag="proj")
                        for kc in range(dc):
                            nc.tensor.matmul(
                                ps,
                                wt_sb[:, kc, ec * 128:(ec + 1) * 128],
                                et[:, kc, :],
                                start=(kc == 0),
                                stop=(kc == dc - 1),
                            )
                        if ec == min(2, dc - 1) and pending is not None:
                            flush_pending(pending)
                            pending = None
                        en = enp.tile([128, st], f32, tag="en")
                        nc.scalar.activation(
                            en, ps, Tanh, bias=bias_all[:, ec, bb:bb + 1]
                        )
                        # v-dot on DVE: acc[p, s] accumulates v[ec*128+p]*en
                        if ec == 0:
                            nc.vector.tensor_scalar_mul(
                                acc, en, vcol_sb[:, 0:1]
                            )
                        else:
                            nc.vector.scalar_tensor_tensor(
                                acc,
                                en,
                                vcol_sb[:, ec:ec + 1],
                                acc,
                                op0=Alu.mult,
                                op1=Alu.add,
                            )
                    if pending is not None:
                        flush_pending(pending)
                    pending = (acc, scores_sb, sti)
                    if bb == 0 and sti == 1 and whT_tiles:
                        # Late "reads" of the Wh^T tiles so their pool slots
                        # (shared with the pass-2 en2 tiles) release only
                        # now -- keeps the en2 prefetch DMAs from competing
                        # with the startup encT/We^T streams for HBM BW.
                        hold = consts.tile([1, 1], f32, name="hold")
                        for whx in whT_tiles:
                            nc.vector.tensor_copy(hold, whx[0:1, 0:1])
                        whT_tiles = []
                if pending is not None:
                    flush_pending(pending)
                    pending = None

                if stage == "p1":
                    nc.gpsimd.dma_start(
                        out=out_d[bb, :].rearrange("(f p) -> p f", p=128),
                        in_=scores_sb,
                    )
                    continue

                # ------------- masked softmax (normalization deferred) ---------
                # exp(score - M) with the global bound M = sum|v| (no
                # max-reduce); mask + per-partition row-sum fused in one
                # DVE pass; all ops are [128, sc2] so they cost ~100 ns.
                attn_raw = sb1.tile([128, sc2], f32, tag="araw")
                nc.scalar.activation(
                    attn_raw, scores_sb, Exp, bias=negM_bc[:, 0:1]
                )
                attn_exp = sb1.tile([128, sc2], f32, tag="aexp")
                psums = sb1.tile([128, 1], f32, tag="psums")
                nc.vector.scalar_tensor_tensor(
                    attn_exp,
                    attn_raw,
                    1.0,
                    validT[bb],
                    op0=Alu.mult,
                    op1=Alu.mult,
                    accum_out=psums,
                )
                # attnT (f32r) is just a rounding copy -- no transpose needed
                attnT = sb1.tile([128, sc2], f32r, tag="attnT")
                nc.scalar.copy(attnT, attn_exp)
                # total sum across partitions -> reciprocal
                psm = psS.tile([128, 1], f32, tag="s", name="psm")
                nc.tensor.matmul(
                    psm[0:1, 0:1], psums, ones_sb[:, 0:1], start=True, stop=True
                )
                if stage == "sm":
                    nc.gpsimd.dma_start(
                        out=out_d[bb, :].rearrange("(f p) -> p f", p=128),
                        in_=attn_exp,
                    )
                    continue
                rinv = sb1.tile([1, 1], f32, tag="rinv")
                nc.vector.reciprocal(rinv, psm[0:1, 0:1])

                # ------------- pass 2: context -------------
                nh = 2 if d > 512 else 1
                hw_ = d // nh
                cps = psM.tile([1, d], f32, tag="m", name="cps")
                for sci in range(sc2):
                    en2 = p2p.tile([128, d], f32r, tag="en2")
                    nc.sync.dma_start(
                        out=en2,
                        in_=enc_d[bb, sci * 128:(sci + 1) * 128, :].bitcast(f32r),
                    )
                    for h in range(nh):
                        nc.tensor.matmul(
                            cps[:, h * hw_:(h + 1) * hw_],
                            attnT[:, sci:sci + 1],
                            en2[:, h * hw_:(h + 1) * hw_],
                            start=(sci == 0),
                            stop=(sci == sc2 - 1),
                        )
                ctx_sb = sb1.tile([1, d], f32, tag="ctx")
                nc.scalar.mul(ctx_sb, cps, rinv[0:1, 0:1])
                nc.gpsimd.dma_start(out=out_d[bb:bb + 1, :], in_=ctx_sb)

    nc.compile()
    return nc


def _get_nc(bl=BL, s=S, d=D, st=512, stage="all"):
    key = (bl, s, d, st, stage)
    if key not in _NC_CACHE:
        _NC_CACHE[key] = _build_program(bl, s, d, st, stage)
    return _NC_CACHE[key]


def _make_in_maps(encoder_outputs, hidden, lengths, W, b, v):
    enc = np.asarray(encoder_outputs, dtype=np.float32)
    hid = np.asarray(hidden, dtype=np.float32)
    len_ = np.asarray(lengths, dtype=np.int32)
    Wn = np.asarray(W, dtype=np.float32)
    bn = np.asarray(b, dtype=np.float32)
    vn = np.asarray(v, dtype=np.float32)

    dc = D // 128
    wt = np.ascontiguousarray(Wn.T)                      # [2D, D]
    bcol = np.ascontiguousarray(bn.reshape(dc, 128).T)   # [128, dc]
    vcol = np.ascontiguousarray(vn.reshape(dc, 128).T)
    in_maps = []
    for i in range(NCORES):
        sl = slice(BL * i, BL * (i + 1))
        e = enc[sl]
        in_maps.append(
            dict(
                encT=np.ascontiguousarray(e.transpose(0, 2, 1)),
                enc=np.ascontiguousarray(e),
                wt=wt,
                hidT=np.ascontiguousarray(hid[sl].T),
                bcol=bcol,
                vcol=vcol,
                len_i=np.ascontiguousarray(
                    np.broadcast_to(len_[sl].reshape(1, BL), (128, BL)).copy()
                ),
            )
        )
    return in_maps


def run(inputs, trace=False):
    """Run on 8 NeuronCores; returns (output [B,1,D], BassKernelResults)."""
    from concourse.bass_utils import run_bass_kernel_spmd

    nc = _get_nc()
    in_maps = _make_in_maps(**inputs)
    r = run_bass_kernel_spmd(
        nc, in_maps, core_ids=list(range(NCORES)), trace=trace
    )
    out = np.concatenate(
        [np.asarray(r.results[i]["ctx_out"]) for i in range(NCORES)], axis=0
    )
    return out[:, None, :].astype(np.float32), r


def kernel(encoder_outputs, hidden, lengths, W, b, v):
    out, _ = run(
        dict(
            encoder_outputs=encoder_outputs,
            hidden=hidden,
            lengths=lengths,
            W=W,
            b=b,
            v=v,
        )
    )
    return out
